# revision 8
# baseline (speedup 1.0000x reference)
"""GNN aggregator (SpMM + bi-interaction MLP) as a Bass/Tile kernel on 8 TRN2 cores.

Strategy (destination sharding, no collectives):
  - Destination rows are packed into 392 windows of <=128 rows (greedy balance of
    per-window edge counts, separately for "lo" sources < 25000 and "hi" sources),
    49 windows per core; each core is fully independent.
  - Edges are routed to their destination window and laid out in 128-edge tiles:
    lo-source tiles first, then hi-source tiles (dma_gather indices are int16, so
    each gather call addresses one half of the node table).
  - Per window: two dma_gather calls fetch the source rows x[cols] (512B per edge);
    a selector matrix S[e, d] = vals[e] * (drel[e] == d) is built on the vector
    engine from an iota tile; side.T = sum_t G_t.T @ S_t accumulates in PSUM.
  - The bi-interaction tail is fused per window: side -> bf16,
    leaky_relu(W1 @ (ego+side) + b1) + leaky_relu(W2 @ (ego*side) + b2),
    in transposed [dim, dest] layout so the 128x128 weights stay stationary.
  - The host pre-permutes ego/edge arrays into window layout and inverts on return.
"""
import heapq

import numpy as np

import concourse.bass as bass
import concourse.bacc as bacc
import concourse.tile as tile
from concourse import mybir
from concourse.bass_utils import run_bass_kernel_spmd

P = 128
D = 128
N_NODES = 50000
NHALF = 25000
N_EDGES = 640000
N_CORES = 8
W_CORE = 49                 # windows per core
W_TOT = W_CORE * N_CORES    # 392
NDEST_CORE = W_CORE * P     # 6272 dest slots per core
GCH = 8                     # max edge tiles per dma_gather call

f32 = mybir.dt.float32
bf16 = mybir.dt.bfloat16
i16 = mybir.dt.int16

_PROGRAM_CACHE: dict = {}


def _pack_windows(rows, lo_mask):
    """Assign destination rows to W_TOT windows (<=128 rows each), balancing both
    lo- and hi-edge counts per window. Returns (loc_of_row, win_of_row,
    window_rows, K_LO, K_HI)."""
    deg_lo = np.bincount(rows[lo_mask], minlength=N_NODES)
    deg_hi = np.bincount(rows[~lo_mask], minlength=N_NODES)
    deg = deg_lo + deg_hi
    order = np.argsort(-deg, kind="stable")
    win_of_row = np.empty(N_NODES, dtype=np.int64)
    loc_of_row = np.empty(N_NODES, dtype=np.int64)
    rowcount = np.zeros(W_TOT, dtype=np.int64)
    lo_load = np.zeros(W_TOT, dtype=np.int64)
    hi_load = np.zeros(W_TOT, dtype=np.int64)
    heap = [(0, 0, wid) for wid in range(W_TOT)]
    heapq.heapify(heap)
    for r in order:
        while True:
            _, _, wid = heapq.heappop(heap)
            if rowcount[wid] < P:
                break
        win_of_row[r] = wid
        loc_of_row[r] = rowcount[wid]
        rowcount[wid] += 1
        lo_load[wid] += int(deg_lo[r])
        hi_load[wid] += int(deg_hi[r])
        if rowcount[wid] < P:
            key = max(lo_load[wid], hi_load[wid])
            heapq.heappush(heap, (key, lo_load[wid] + hi_load[wid], wid))
    window_rows = np.full((W_TOT, P), -1, dtype=np.int64)
    window_rows[win_of_row, loc_of_row] = np.arange(N_NODES)
    K_LO = int(np.ceil(lo_load.max() / P))
    K_HI = int(np.ceil(hi_load.max() / P))
    return win_of_row, loc_of_row, window_rows, K_LO, K_HI


def _build_program(K_LO: int, K_HI: int):
    """One SPMD program shared by all 8 cores."""
    key = (K_LO, K_HI)
    if key in _PROGRAM_CACHE:
        return _PROGRAM_CACHE[key]

    T2 = K_LO + K_HI
    WT = W_CORE * T2
    # consts columns: drel | vals | iota | W1T | W2T | b1 | b2
    CW = 2 * WT + 3 * P + 2

    nc = bacc.Bacc()
    x_d = nc.declare_dram_parameter("x_full", [N_NODES, D], f32, isOutput=False)
    ego_d = nc.declare_dram_parameter("ego_T", [D, NDEST_CORE], f32, isOutput=False)
    idx_d = nc.declare_dram_parameter("idx16", [P, WT * 8], i16, isOutput=False)
    consts_d = nc.declare_dram_parameter("consts", [P, CW], f32, isOutput=False)
    out_d = nc.declare_dram_parameter("out_T", [D, NDEST_CORE], f32, isOutput=True)

    with tile.TileContext(nc) as tc:
        with tc.tile_pool(name="const", bufs=1) as cpool, \
             tc.tile_pool(name="g", bufs=3) as gpool, \
             tc.tile_pool(name="s", bufs=6) as spool, \
             tc.tile_pool(name="mid", bufs=3) as mpool, \
             tc.tile_pool(name="o", bufs=3) as opool, \
             tc.tile_pool(name="psA", bufs=2, space="PSUM") as psA, \
             tc.tile_pool(name="psB", bufs=2, space="PSUM") as psB:

            ego_sb = cpool.tile([D, NDEST_CORE], f32)
            idx_sb = cpool.tile([P, WT * 8], i16)
            consts_sb = cpool.tile([P, CW], f32)

            nc.sync.dma_start(out=idx_sb[:], in_=idx_d[:])
            nc.sync.dma_start(out=consts_sb[:], in_=consts_d[:])
            nc.sync.dma_start(out=ego_sb[:], in_=ego_d[:])

            iota_sb = consts_sb[:, 2 * WT : 2 * WT + P]
            w1_sb = consts_sb[:, 2 * WT + P : 2 * WT + 2 * P]
            w2_sb = consts_sb[:, 2 * WT + 2 * P : 2 * WT + 3 * P]
            b1_sb = consts_sb[:, 2 * WT + 3 * P : 2 * WT + 3 * P + 1]
            b2_sb = consts_sb[:, 2 * WT + 3 * P + 1 : 2 * WT + 3 * P + 2]

            for w in range(W_CORE):
                G = gpool.tile([P, T2, D], f32, tag="G")
                # segments: (tile offset, n tiles, table base row)
                for seg_t0, seg_k, base in ((0, K_LO, 0), (K_LO, K_HI, NHALF)):
                    t = seg_t0
                    while t < seg_t0 + seg_k:
                        k = min(GCH, seg_t0 + seg_k - t)
                        nc.gpsimd.dma_gather(
                            G[:, t : t + k, :],
                            x_d[base:, :],
                            idx_sb[:, (w * T2 + t) * 8 : (w * T2 + t + k) * 8],
                            k * P,
                            k * P,
                            D,
                        )
                        t += k
                ps = psA.tile([P, P], f32)
                for t in range(T2):
                    S = spool.tile([P, P], f32, tag="S")
                    c = w * T2 + t
                    nc.vector.tensor_scalar(
                        out=S[:],
                        in0=iota_sb,
                        scalar1=consts_sb[:, c : c + 1],
                        scalar2=consts_sb[:, WT + c : WT + c + 1],
                        op0=mybir.AluOpType.is_equal,
                        op1=mybir.AluOpType.mult,
                    )
                    nc.tensor.matmul(
                        ps[:],
                        lhsT=G[:, t, :],
                        rhs=S[:],
                        start=(t == 0),
                        stop=(t == T2 - 1),
                    )
                side_bf_t = mpool.tile([P, P], bf16, tag="sidebf")
                nc.vector.tensor_copy(out=side_bf_t[:], in_=ps[:])
                sum_in = mpool.tile([P, P], f32, tag="sumin")
                nc.vector.tensor_tensor(
                    out=sum_in[:],
                    in0=ego_sb[:, w * P : (w + 1) * P],
                    in1=side_bf_t[:],
                    op=mybir.AluOpType.add,
                )
                prod_in = mpool.tile([P, P], f32, tag="prodin")
                nc.vector.tensor_tensor(
                    out=prod_in[:],
                    in0=ego_sb[:, w * P : (w + 1) * P],
                    in1=side_bf_t[:],
                    op=mybir.AluOpType.mult,
                )
                p1 = psB.tile([P, P], f32, tag="p1")
                nc.tensor.matmul(p1[:], lhsT=w1_sb, rhs=sum_in[:], start=True, stop=True)
                p2 = psB.tile([P, P], f32, tag="p2")
                nc.tensor.matmul(p2[:], lhsT=w2_sb, rhs=prod_in[:], start=True, stop=True)
                o1 = opool.tile([P, P], f32, tag="o1")
                nc.scalar.activation(
                    out=o1[:], in_=p1[:],
                    func=mybir.ActivationFunctionType.Lrelu,
                    bias=b1_sb, scale=1.0, alpha=0.01,
                )
                o2 = opool.tile([P, P], f32, tag="o2")
                nc.scalar.activation(
                    out=o2[:], in_=p2[:],
                    func=mybir.ActivationFunctionType.Lrelu,
                    bias=b2_sb, scale=1.0, alpha=0.01,
                )
                oo = opool.tile([P, P], f32, tag="oo")
                nc.vector.tensor_tensor(
                    out=oo[:], in0=o1[:], in1=o2[:], op=mybir.AluOpType.add
                )
                nc.sync.dma_start(out=out_d[:, w * P : (w + 1) * P], in_=oo[:])

    nc.finalize()
    _PROGRAM_CACHE[key] = nc
    return nc


def _prepare_inputs(ego_embeddings, vals, W1, b1, W2, b2, rows, cols):
    rows = np.asarray(rows).astype(np.int64)
    cols = np.asarray(cols).astype(np.int64)
    vals = np.asarray(vals, dtype=np.float32)
    ego = np.ascontiguousarray(np.asarray(ego_embeddings, dtype=np.float32))

    lo_mask = cols < NHALF
    win_of_row, loc_of_row, window_rows, K_LO, K_HI = _pack_windows(rows, lo_mask)
    T2 = K_LO + K_HI
    WT_TOT = W_TOT * T2

    # linear slot within each window: lo edges first (0..), hi edges from K_LO*P
    w_of_e = win_of_row[rows]
    grp = w_of_e * 2 + (~lo_mask).astype(np.int64)
    order_e = np.argsort(grp, kind="stable")
    ge = grp[order_e]
    counts = np.bincount(ge, minlength=W_TOT * 2)
    starts = np.concatenate([[0], np.cumsum(counts)[:-1]])
    rank = np.arange(N_EDGES) - starts[ge]
    slot = rank + np.where(ge % 2 == 1, K_LO * P, 0)
    we = ge // 2
    tile_of = slot // P
    part_of = slot % P
    colpos = we * T2 + tile_of

    vals_arr = np.zeros((P, WT_TOT), dtype=np.float32)
    drel_arr = np.zeros((P, WT_TOT), dtype=np.float32)
    vals_arr[part_of, colpos] = vals[order_e]
    drel_arr[part_of, colpos] = loc_of_row[rows[order_e]]

    # int16 gather indices: wrapped [16, n/16] within each window segment,
    # replicated across the 8 16-partition groups
    rel = np.where(lo_mask, cols, cols - NHALF).astype(np.int64)[order_e]
    wrapped = np.zeros((16, WT_TOT * 8), dtype=np.int16)
    wrapped[slot % 16, we * T2 * 8 + slot // 16] = rel
    idx16 = np.tile(wrapped, (8, 1))

    perm = window_rows.ravel()
    valid = perm >= 0
    ego_T_full = np.zeros((D, W_TOT * P), dtype=np.float32)
    ego_T_full[:, valid] = ego[perm[valid]].T

    iota_np = np.ascontiguousarray(
        np.broadcast_to(np.arange(P, dtype=np.float32), (P, P))
    )
    W1T = np.ascontiguousarray(np.asarray(W1, dtype=np.float32).T)
    W2T = np.ascontiguousarray(np.asarray(W2, dtype=np.float32).T)
    b1c = np.ascontiguousarray(np.asarray(b1, dtype=np.float32)[:, None])
    b2c = np.ascontiguousarray(np.asarray(b2, dtype=np.float32)[:, None])

    in_maps = []
    for c in range(N_CORES):
        wlo, whi = c * W_CORE, (c + 1) * W_CORE
        consts = np.concatenate(
            [
                drel_arr[:, wlo * T2 : whi * T2],
                vals_arr[:, wlo * T2 : whi * T2],
                iota_np,
                W1T,
                W2T,
                b1c,
                b2c,
            ],
            axis=1,
        )
        in_maps.append(
            {
                "x_full": ego,
                "ego_T": np.ascontiguousarray(ego_T_full[:, wlo * P : whi * P]),
                "idx16": np.ascontiguousarray(idx16[:, wlo * T2 * 8 : whi * T2 * 8]),
                "consts": np.ascontiguousarray(consts),
            }
        )
    return K_LO, K_HI, in_maps, perm, valid


def kernel(ego_embeddings, vals, W1, b1, W2, b2, rows, cols, _trace=False):
    K_LO, K_HI, in_maps, perm, valid = _prepare_inputs(
        ego_embeddings, vals, W1, b1, W2, b2, rows, cols
    )
    nc = _build_program(K_LO, K_HI)
    res = run_bass_kernel_spmd(nc, in_maps, list(range(N_CORES)), trace=_trace)
    out_T = np.concatenate([r["out_T"] for r in res.results], axis=1)
    out = np.empty((N_NODES, D), dtype=np.float32)
    out[perm[valid]] = out_T.T[valid]
    if _trace:
        kernel.last_exec_time_ns = res.exec_time_ns
        kernel.last_results = res
    return out


# revision 9
# speedup vs baseline: 1.4361x; 1.4361x over previous
"""GNN aggregator (SpMM + bi-interaction MLP) as a Bass/Tile kernel on 8 TRN2 cores.

Strategy (destination sharding, no collectives):
  - Destination rows are packed into 392 windows of <=128 rows (greedy balance of
    per-window edge counts, separately for "lo" sources < 25000 and "hi" sources),
    49 windows per core; each core is fully independent.
  - Edges are routed to their destination window and laid out in 128-edge tiles:
    lo-source tiles first, then hi-source tiles (dma_gather indices are int16, so
    each gather call addresses one half of the node table).
  - Per window: two dma_gather calls fetch the source rows x[cols] (512B per edge);
    a selector matrix S[e, d] = vals[e] * (drel[e] == d) is built on the vector
    engine from an iota tile; side.T = sum_t G_t.T @ S_t accumulates in PSUM.
  - The bi-interaction tail is fused per window: side -> bf16,
    leaky_relu(W1 @ (ego+side) + b1) + leaky_relu(W2 @ (ego*side) + b2),
    in transposed [dim, dest] layout so the 128x128 weights stay stationary.
  - The host pre-permutes ego/edge arrays into window layout and inverts on return.
"""
import heapq

import numpy as np

import concourse.bass as bass
import concourse.bacc as bacc
import concourse.tile as tile
from concourse import mybir
from concourse.bass_utils import run_bass_kernel_spmd

P = 128
D = 128
N_NODES = 50000
NHALF = 25000
N_EDGES = 640000
N_CORES = 8
W_CORE = 49                 # windows per core
W_TOT = W_CORE * N_CORES    # 392
NDEST_CORE = W_CORE * P     # 6272 dest slots per core
GCH = 8                     # max edge tiles per dma_gather call

f32 = mybir.dt.float32
bf16 = mybir.dt.bfloat16
i16 = mybir.dt.int16

_PROGRAM_CACHE: dict = {}
BF16 = False          # scatter path (gather table + selector + matmul) in bf16
QUEUE_ROT = (1, 2, 3, 0)  # async queues first; queue 0 dispatches synchronously


def _pack_windows(rows, lo_mask):
    """Assign destination rows to W_TOT windows (<=128 rows each), balancing both
    lo- and hi-edge counts per window. Returns (loc_of_row, win_of_row,
    window_rows, K_LO, K_HI)."""
    deg_lo = np.bincount(rows[lo_mask], minlength=N_NODES)
    deg_hi = np.bincount(rows[~lo_mask], minlength=N_NODES)
    deg = deg_lo + deg_hi
    order = np.argsort(-deg, kind="stable")
    win_of_row = np.empty(N_NODES, dtype=np.int64)
    loc_of_row = np.empty(N_NODES, dtype=np.int64)
    rowcount = np.zeros(W_TOT, dtype=np.int64)
    lo_load = np.zeros(W_TOT, dtype=np.int64)
    hi_load = np.zeros(W_TOT, dtype=np.int64)
    heap = [(0, 0, wid) for wid in range(W_TOT)]
    heapq.heapify(heap)
    for r in order:
        while True:
            _, _, wid = heapq.heappop(heap)
            if rowcount[wid] < P:
                break
        win_of_row[r] = wid
        loc_of_row[r] = rowcount[wid]
        rowcount[wid] += 1
        lo_load[wid] += int(deg_lo[r])
        hi_load[wid] += int(deg_hi[r])
        if rowcount[wid] < P:
            key = max(lo_load[wid], hi_load[wid])
            heapq.heappush(heap, (key, lo_load[wid] + hi_load[wid], wid))
    window_rows = np.full((W_TOT, P), -1, dtype=np.int64)
    window_rows[win_of_row, loc_of_row] = np.arange(N_NODES)
    K_LO = int(np.ceil(lo_load.max() / P))
    K_HI = int(np.ceil(hi_load.max() / P))
    return win_of_row, loc_of_row, window_rows, K_LO, K_HI


def _build_program(K_LO: int, K_HI: int):
    """One SPMD program shared by all 8 cores."""
    key = (K_LO, K_HI, BF16)
    if key in _PROGRAM_CACHE:
        return _PROGRAM_CACHE[key]

    T2 = K_LO + K_HI
    WT = W_CORE * T2
    # consts columns: drel | vals | iota | W1T | W2T | b1 | b2
    CW = 2 * WT + 3 * P + 2

    nc = bacc.Bacc(num_swdge_queues=4)
    gdt = bf16 if BF16 else f32
    x_d = nc.declare_dram_parameter("x_full", [N_NODES, D], gdt, isOutput=False)
    ego_d = nc.declare_dram_parameter("ego_T", [D, NDEST_CORE], f32, isOutput=False)
    idx_d = nc.declare_dram_parameter("idx16", [P, WT * 8], i16, isOutput=False)
    consts_d = nc.declare_dram_parameter("consts", [P, CW], f32, isOutput=False)
    out_d = nc.declare_dram_parameter("out_T", [D, NDEST_CORE], f32, isOutput=True)

    with tile.TileContext(nc) as tc:
        with tc.tile_pool(name="const", bufs=1) as cpool, \
             tc.tile_pool(name="g", bufs=3) as gpool, \
             tc.tile_pool(name="s", bufs=6) as spool, \
             tc.tile_pool(name="mid", bufs=3) as mpool, \
             tc.tile_pool(name="o", bufs=3) as opool, \
             tc.tile_pool(name="psA", bufs=2, space="PSUM") as psA, \
             tc.tile_pool(name="psB", bufs=2, space="PSUM") as psB:

            ego_sb = cpool.tile([D, NDEST_CORE], f32)
            idx_sb = cpool.tile([P, WT * 8], i16)
            consts_sb = cpool.tile([P, CW], f32)

            nc.sync.dma_start(out=idx_sb[:], in_=idx_d[:])
            nc.sync.dma_start(out=consts_sb[:], in_=consts_d[:])
            nc.sync.dma_start(out=ego_sb[:], in_=ego_d[:])

            iota_sb = consts_sb[:, 2 * WT : 2 * WT + P]
            if BF16:
                iota_g_sb = cpool.tile([P, P], bf16)
                nc.vector.tensor_copy(out=iota_g_sb[:], in_=iota_sb)
            else:
                iota_g_sb = iota_sb
            w1_sb = consts_sb[:, 2 * WT + P : 2 * WT + 2 * P]
            w2_sb = consts_sb[:, 2 * WT + 2 * P : 2 * WT + 3 * P]
            b1_sb = consts_sb[:, 2 * WT + 3 * P : 2 * WT + 3 * P + 1]
            b2_sb = consts_sb[:, 2 * WT + 3 * P + 1 : 2 * WT + 3 * P + 2]

            qn = 0
            for w in range(W_CORE):
                G = gpool.tile([P, T2, D], gdt, tag="G")
                # segments: (tile offset, n tiles, table base row)
                for seg_t0, seg_k, base in ((0, K_LO, 0), (K_LO, K_HI, NHALF)):
                    t = seg_t0
                    while t < seg_t0 + seg_k:
                        k = min(GCH, seg_t0 + seg_k - t)
                        nc.gpsimd.dma_gather(
                            G[:, t : t + k, :],
                            x_d[base:, :],
                            idx_sb[:, (w * T2 + t) * 8 : (w * T2 + t + k) * 8],
                            k * P,
                            k * P,
                            D,
                            queue_num=QUEUE_ROT[qn % 4],
                        )
                        qn += 1
                        t += k
                ps = psA.tile([P, P], f32)
                for t in range(T2):
                    S = spool.tile([P, P], gdt, tag="S")
                    c = w * T2 + t
                    nc.vector.tensor_scalar(
                        out=S[:],
                        in0=iota_g_sb,
                        scalar1=consts_sb[:, c : c + 1],
                        scalar2=consts_sb[:, WT + c : WT + c + 1],
                        op0=mybir.AluOpType.is_equal,
                        op1=mybir.AluOpType.mult,
                    )
                    nc.tensor.matmul(
                        ps[:],
                        lhsT=G[:, t, :],
                        rhs=S[:],
                        start=(t == 0),
                        stop=(t == T2 - 1),
                    )
                side_bf_t = mpool.tile([P, P], bf16, tag="sidebf")
                nc.vector.tensor_copy(out=side_bf_t[:], in_=ps[:])
                sum_in = mpool.tile([P, P], f32, tag="sumin")
                nc.vector.tensor_tensor(
                    out=sum_in[:],
                    in0=ego_sb[:, w * P : (w + 1) * P],
                    in1=side_bf_t[:],
                    op=mybir.AluOpType.add,
                )
                prod_in = mpool.tile([P, P], f32, tag="prodin")
                nc.vector.tensor_tensor(
                    out=prod_in[:],
                    in0=ego_sb[:, w * P : (w + 1) * P],
                    in1=side_bf_t[:],
                    op=mybir.AluOpType.mult,
                )
                p1 = psB.tile([P, P], f32, tag="p1")
                nc.tensor.matmul(p1[:], lhsT=w1_sb, rhs=sum_in[:], start=True, stop=True)
                p2 = psB.tile([P, P], f32, tag="p2")
                nc.tensor.matmul(p2[:], lhsT=w2_sb, rhs=prod_in[:], start=True, stop=True)
                o1 = opool.tile([P, P], f32, tag="o1")
                nc.scalar.activation(
                    out=o1[:], in_=p1[:],
                    func=mybir.ActivationFunctionType.Lrelu,
                    bias=b1_sb, scale=1.0, alpha=0.01,
                )
                o2 = opool.tile([P, P], f32, tag="o2")
                nc.scalar.activation(
                    out=o2[:], in_=p2[:],
                    func=mybir.ActivationFunctionType.Lrelu,
                    bias=b2_sb, scale=1.0, alpha=0.01,
                )
                oo = opool.tile([P, P], f32, tag="oo")
                nc.vector.tensor_tensor(
                    out=oo[:], in0=o1[:], in1=o2[:], op=mybir.AluOpType.add
                )
                nc.sync.dma_start(out=out_d[:, w * P : (w + 1) * P], in_=oo[:])

    nc.finalize()
    _PROGRAM_CACHE[key] = nc
    return nc


def _prepare_inputs(ego_embeddings, vals, W1, b1, W2, b2, rows, cols):
    rows = np.asarray(rows).astype(np.int64)
    cols = np.asarray(cols).astype(np.int64)
    vals = np.asarray(vals, dtype=np.float32)
    ego = np.ascontiguousarray(np.asarray(ego_embeddings, dtype=np.float32))

    lo_mask = cols < NHALF
    win_of_row, loc_of_row, window_rows, K_LO, K_HI = _pack_windows(rows, lo_mask)
    T2 = K_LO + K_HI
    WT_TOT = W_TOT * T2

    # linear slot within each window: lo edges first (0..), hi edges from K_LO*P
    w_of_e = win_of_row[rows]
    grp = w_of_e * 2 + (~lo_mask).astype(np.int64)
    order_e = np.argsort(grp, kind="stable")
    ge = grp[order_e]
    counts = np.bincount(ge, minlength=W_TOT * 2)
    starts = np.concatenate([[0], np.cumsum(counts)[:-1]])
    rank = np.arange(N_EDGES) - starts[ge]
    slot = rank + np.where(ge % 2 == 1, K_LO * P, 0)
    we = ge // 2
    tile_of = slot // P
    part_of = slot % P
    colpos = we * T2 + tile_of

    vals_arr = np.zeros((P, WT_TOT), dtype=np.float32)
    drel_arr = np.zeros((P, WT_TOT), dtype=np.float32)
    vals_arr[part_of, colpos] = vals[order_e]
    drel_arr[part_of, colpos] = loc_of_row[rows[order_e]]

    # int16 gather indices: wrapped [16, n/16] within each window segment,
    # replicated across the 8 16-partition groups
    rel = np.where(lo_mask, cols, cols - NHALF).astype(np.int64)[order_e]
    wrapped = np.zeros((16, WT_TOT * 8), dtype=np.int16)
    wrapped[slot % 16, we * T2 * 8 + slot // 16] = rel
    idx16 = np.tile(wrapped, (8, 1))

    perm = window_rows.ravel()
    valid = perm >= 0
    ego_T_full = np.zeros((D, W_TOT * P), dtype=np.float32)
    ego_T_full[:, valid] = ego[perm[valid]].T

    iota_np = np.ascontiguousarray(
        np.broadcast_to(np.arange(P, dtype=np.float32), (P, P))
    )
    W1T = np.ascontiguousarray(np.asarray(W1, dtype=np.float32).T)
    W2T = np.ascontiguousarray(np.asarray(W2, dtype=np.float32).T)
    b1c = np.ascontiguousarray(np.asarray(b1, dtype=np.float32)[:, None])
    b2c = np.ascontiguousarray(np.asarray(b2, dtype=np.float32)[:, None])

    import ml_dtypes
    x_tab = (np.ascontiguousarray(ego.astype(ml_dtypes.bfloat16))
             if BF16 else ego)
    in_maps = []
    for c in range(N_CORES):
        wlo, whi = c * W_CORE, (c + 1) * W_CORE
        consts = np.concatenate(
            [
                drel_arr[:, wlo * T2 : whi * T2],
                vals_arr[:, wlo * T2 : whi * T2],
                iota_np,
                W1T,
                W2T,
                b1c,
                b2c,
            ],
            axis=1,
        )
        in_maps.append(
            {
                "x_full": x_tab,
                "ego_T": np.ascontiguousarray(ego_T_full[:, wlo * P : whi * P]),
                "idx16": np.ascontiguousarray(idx16[:, wlo * T2 * 8 : whi * T2 * 8]),
                "consts": np.ascontiguousarray(consts),
            }
        )
    return K_LO, K_HI, in_maps, perm, valid


def kernel(ego_embeddings, vals, W1, b1, W2, b2, rows, cols, _trace=False):
    K_LO, K_HI, in_maps, perm, valid = _prepare_inputs(
        ego_embeddings, vals, W1, b1, W2, b2, rows, cols
    )
    nc = _build_program(K_LO, K_HI)
    res = run_bass_kernel_spmd(nc, in_maps, list(range(N_CORES)), trace=_trace)
    out_T = np.concatenate([r["out_T"] for r in res.results], axis=1)
    out = np.empty((N_NODES, D), dtype=np.float32)
    out[perm[valid]] = out_T.T[valid]
    if _trace:
        kernel.last_exec_time_ns = res.exec_time_ns
        kernel.last_results = res
    return out


# revision 10
# speedup vs baseline: 2.9659x; 2.0653x over previous
"""GNN aggregator (SpMM + bi-interaction MLP) as a Bass/Tile kernel on 8 TRN2 cores.

Strategy (destination sharding, no collectives):
  - Destination rows are packed into 392 windows of <=128 rows (greedy balance of
    per-window edge counts, separately for "lo" sources < 25000 and "hi" sources),
    49 windows per core; each core is fully independent.
  - Edges are routed to their destination window and laid out in 128-edge tiles:
    lo-source tiles first, then hi-source tiles (dma_gather indices are int16, so
    each gather call addresses one half of the node table).
  - Per window: two dma_gather calls fetch the source rows x[cols] (512B per edge);
    a selector matrix S[e, d] = vals[e] * (drel[e] == d) is built on the vector
    engine from an iota tile; side.T = sum_t G_t.T @ S_t accumulates in PSUM.
  - The bi-interaction tail is fused per window: side -> bf16,
    leaky_relu(W1 @ (ego+side) + b1) + leaky_relu(W2 @ (ego*side) + b2),
    in transposed [dim, dest] layout so the 128x128 weights stay stationary.
  - The host pre-permutes ego/edge arrays into window layout and inverts on return.
"""
import heapq

import numpy as np

import concourse.bass as bass
import concourse.bacc as bacc
import concourse.tile as tile
from concourse import mybir
from concourse.bass_utils import run_bass_kernel_spmd

P = 128
D = 128
N_NODES = 50000
NHALF = 25000
N_EDGES = 640000
N_CORES = 8
W_CORE = 49                 # windows per core
W_TOT = W_CORE * N_CORES    # 392
NDEST_CORE = W_CORE * P     # 6272 dest slots per core
GCH = 8                     # max edge tiles per dma_gather call

f32 = mybir.dt.float32
bf16 = mybir.dt.bfloat16
i16 = mybir.dt.int16

_PROGRAM_CACHE: dict = {}
BF16 = False          # scatter path (gather table + selector + matmul) in bf16
QUEUE_ROT = (1, 2, 3, 0)  # async queues first; queue 0 dispatches synchronously


def _pack_windows(rows, lo_mask):
    """Assign destination rows to W_TOT windows (<=128 rows each), balancing both
    lo- and hi-edge counts per window. Returns (loc_of_row, win_of_row,
    window_rows, K_LO, K_HI)."""
    deg_lo = np.bincount(rows[lo_mask], minlength=N_NODES)
    deg_hi = np.bincount(rows[~lo_mask], minlength=N_NODES)
    deg = deg_lo + deg_hi
    order = np.argsort(-deg, kind="stable")
    win_of_row = np.empty(N_NODES, dtype=np.int64)
    loc_of_row = np.empty(N_NODES, dtype=np.int64)
    rowcount = np.zeros(W_TOT, dtype=np.int64)
    lo_load = np.zeros(W_TOT, dtype=np.int64)
    hi_load = np.zeros(W_TOT, dtype=np.int64)
    heap = [(0, 0, wid) for wid in range(W_TOT)]
    heapq.heapify(heap)
    for r in order:
        while True:
            _, _, wid = heapq.heappop(heap)
            if rowcount[wid] < P:
                break
        win_of_row[r] = wid
        loc_of_row[r] = rowcount[wid]
        rowcount[wid] += 1
        lo_load[wid] += int(deg_lo[r])
        hi_load[wid] += int(deg_hi[r])
        if rowcount[wid] < P:
            key = max(lo_load[wid], hi_load[wid])
            heapq.heappush(heap, (key, lo_load[wid] + hi_load[wid], wid))
    window_rows = np.full((W_TOT, P), -1, dtype=np.int64)
    window_rows[win_of_row, loc_of_row] = np.arange(N_NODES)
    K_LO = int(np.ceil(lo_load.max() / P))
    K_HI = int(np.ceil(hi_load.max() / P))
    return win_of_row, loc_of_row, window_rows, K_LO, K_HI


def _build_program(K_LO: int, K_HI: int):
    """One SPMD program shared by all 8 cores."""
    key = (K_LO, K_HI, BF16)
    if key in _PROGRAM_CACHE:
        return _PROGRAM_CACHE[key]

    T2 = K_LO + K_HI
    WT = W_CORE * T2
    # consts columns: drel | vals | iota | W1T | W2T | b1 | b2
    CW = 2 * WT + 3 * P + 2

    nc = bacc.Bacc(num_swdge_queues=4)
    gdt = bf16 if BF16 else f32
    x_d = nc.declare_dram_parameter("x_full", [N_NODES, D], gdt, isOutput=False)
    ego_d = nc.declare_dram_parameter("ego_T", [D, NDEST_CORE], f32, isOutput=False)
    idx_d = nc.declare_dram_parameter("idx16", [P, WT * 8], i16, isOutput=False)
    consts_d = nc.declare_dram_parameter("consts", [P, CW], f32, isOutput=False)
    out_d = nc.declare_dram_parameter("out_T", [D, NDEST_CORE], f32, isOutput=True)

    with tile.TileContext(nc) as tc:
        with tc.tile_pool(name="const", bufs=1) as cpool, \
             tc.tile_pool(name="g", bufs=3) as gpool, \
             tc.tile_pool(name="s", bufs=3) as spool, \
             tc.tile_pool(name="mid", bufs=3) as mpool, \
             tc.tile_pool(name="o", bufs=3) as opool, \
             tc.tile_pool(name="psA", bufs=2, space="PSUM") as psA, \
             tc.tile_pool(name="psB", bufs=2, space="PSUM") as psB:

            ego_sb = cpool.tile([D, NDEST_CORE], f32)
            idx_sb = cpool.tile([P, WT * 8], i16)
            consts_sb = cpool.tile([P, CW], f32)

            nc.sync.dma_start(out=idx_sb[:], in_=idx_d[:])
            nc.sync.dma_start(out=consts_sb[:], in_=consts_d[:])
            nc.sync.dma_start(out=ego_sb[:], in_=ego_d[:])

            iota_sb = consts_sb[:, 2 * WT : 2 * WT + P]
            if BF16:
                iota_g_sb = cpool.tile([P, P], bf16)
                nc.vector.tensor_copy(out=iota_g_sb[:], in_=iota_sb)
            else:
                iota_g_sb = iota_sb
            w1_sb = consts_sb[:, 2 * WT + P : 2 * WT + 2 * P]
            w2_sb = consts_sb[:, 2 * WT + 2 * P : 2 * WT + 3 * P]
            b1_sb = consts_sb[:, 2 * WT + 3 * P : 2 * WT + 3 * P + 1]
            b2_sb = consts_sb[:, 2 * WT + 3 * P + 1 : 2 * WT + 3 * P + 2]

            qn = 0
            for w in range(W_CORE):
                G = gpool.tile([P, T2, D], gdt, tag="G")
                # segments: (tile offset, n tiles, table base row)
                for seg_t0, seg_k, base in ((0, K_LO, 0), (K_LO, K_HI, NHALF)):
                    t = seg_t0
                    while t < seg_t0 + seg_k:
                        k = min(GCH, seg_t0 + seg_k - t)
                        nc.gpsimd.dma_gather(
                            G[:, t : t + k, :],
                            x_d[base:, :],
                            idx_sb[:, (w * T2 + t) * 8 : (w * T2 + t + k) * 8],
                            k * P,
                            k * P,
                            D,
                            queue_num=QUEUE_ROT[qn % 4],
                        )
                        qn += 1
                        t += k
                ps = psA.tile([P, P], f32)
                iota_w = iota_g_sb[:, None, :].to_broadcast([P, T2, P])
                drel_b = consts_sb[:, w * T2 : (w + 1) * T2, None].to_broadcast(
                    [P, T2, P]
                )
                vals_b = consts_sb[:, WT + w * T2 : WT + (w + 1) * T2, None
                                   ].to_broadcast([P, T2, P])
                EQ = spool.tile([P, T2, P], gdt, tag="EQ")
                nc.vector.tensor_tensor(
                    out=EQ[:], in0=iota_w, in1=drel_b, op=mybir.AluOpType.is_equal
                )
                Sw = spool.tile([P, T2, P], gdt, tag="Sw")
                nc.vector.tensor_tensor(
                    out=Sw[:], in0=EQ[:], in1=vals_b, op=mybir.AluOpType.mult
                )
                for t in range(T2):
                    nc.tensor.matmul(
                        ps[:],
                        lhsT=G[:, t, :],
                        rhs=Sw[:, t, :],
                        start=(t == 0),
                        stop=(t == T2 - 1),
                    )
                side_bf_t = mpool.tile([P, P], bf16, tag="sidebf")
                nc.scalar.activation(
                    out=side_bf_t[:], in_=ps[:],
                    func=mybir.ActivationFunctionType.Copy,
                )
                sum_in = mpool.tile([P, P], f32, tag="sumin")
                nc.vector.tensor_tensor(
                    out=sum_in[:],
                    in0=ego_sb[:, w * P : (w + 1) * P],
                    in1=side_bf_t[:],
                    op=mybir.AluOpType.add,
                )
                prod_in = mpool.tile([P, P], f32, tag="prodin")
                nc.vector.tensor_tensor(
                    out=prod_in[:],
                    in0=ego_sb[:, w * P : (w + 1) * P],
                    in1=side_bf_t[:],
                    op=mybir.AluOpType.mult,
                )
                p1 = psB.tile([P, P], f32, tag="p1")
                nc.tensor.matmul(p1[:], lhsT=w1_sb, rhs=sum_in[:], start=True, stop=True)
                p2 = psB.tile([P, P], f32, tag="p2")
                nc.tensor.matmul(p2[:], lhsT=w2_sb, rhs=prod_in[:], start=True, stop=True)
                o1 = opool.tile([P, P], f32, tag="o1")
                nc.scalar.activation(
                    out=o1[:], in_=p1[:],
                    func=mybir.ActivationFunctionType.Lrelu,
                    bias=b1_sb, scale=1.0, alpha=0.01,
                )
                o2 = opool.tile([P, P], f32, tag="o2")
                nc.scalar.activation(
                    out=o2[:], in_=p2[:],
                    func=mybir.ActivationFunctionType.Lrelu,
                    bias=b2_sb, scale=1.0, alpha=0.01,
                )
                oo = opool.tile([P, P], f32, tag="oo")
                nc.vector.tensor_tensor(
                    out=oo[:], in0=o1[:], in1=o2[:], op=mybir.AluOpType.add
                )
                nc.sync.dma_start(out=out_d[:, w * P : (w + 1) * P], in_=oo[:])

    nc.finalize()
    _PROGRAM_CACHE[key] = nc
    return nc


def _prepare_inputs(ego_embeddings, vals, W1, b1, W2, b2, rows, cols):
    rows = np.asarray(rows).astype(np.int64)
    cols = np.asarray(cols).astype(np.int64)
    vals = np.asarray(vals, dtype=np.float32)
    ego = np.ascontiguousarray(np.asarray(ego_embeddings, dtype=np.float32))

    lo_mask = cols < NHALF
    win_of_row, loc_of_row, window_rows, K_LO, K_HI = _pack_windows(rows, lo_mask)
    T2 = K_LO + K_HI
    WT_TOT = W_TOT * T2

    # linear slot within each window: lo edges first (0..), hi edges from K_LO*P
    w_of_e = win_of_row[rows]
    grp = w_of_e * 2 + (~lo_mask).astype(np.int64)
    order_e = np.argsort(grp, kind="stable")
    ge = grp[order_e]
    counts = np.bincount(ge, minlength=W_TOT * 2)
    starts = np.concatenate([[0], np.cumsum(counts)[:-1]])
    rank = np.arange(N_EDGES) - starts[ge]
    slot = rank + np.where(ge % 2 == 1, K_LO * P, 0)
    we = ge // 2
    tile_of = slot // P
    part_of = slot % P
    colpos = we * T2 + tile_of

    vals_arr = np.zeros((P, WT_TOT), dtype=np.float32)
    drel_arr = np.zeros((P, WT_TOT), dtype=np.float32)
    vals_arr[part_of, colpos] = vals[order_e]
    drel_arr[part_of, colpos] = loc_of_row[rows[order_e]]

    # int16 gather indices: wrapped [16, n/16] within each window segment,
    # replicated across the 8 16-partition groups
    rel = np.where(lo_mask, cols, cols - NHALF).astype(np.int64)[order_e]
    wrapped = np.zeros((16, WT_TOT * 8), dtype=np.int16)
    wrapped[slot % 16, we * T2 * 8 + slot // 16] = rel
    idx16 = np.tile(wrapped, (8, 1))

    perm = window_rows.ravel()
    valid = perm >= 0
    ego_T_full = np.zeros((D, W_TOT * P), dtype=np.float32)
    ego_T_full[:, valid] = ego[perm[valid]].T

    iota_np = np.ascontiguousarray(
        np.broadcast_to(np.arange(P, dtype=np.float32), (P, P))
    )
    W1T = np.ascontiguousarray(np.asarray(W1, dtype=np.float32).T)
    W2T = np.ascontiguousarray(np.asarray(W2, dtype=np.float32).T)
    b1c = np.ascontiguousarray(np.asarray(b1, dtype=np.float32)[:, None])
    b2c = np.ascontiguousarray(np.asarray(b2, dtype=np.float32)[:, None])

    import ml_dtypes
    x_tab = (np.ascontiguousarray(ego.astype(ml_dtypes.bfloat16))
             if BF16 else ego)
    in_maps = []
    for c in range(N_CORES):
        wlo, whi = c * W_CORE, (c + 1) * W_CORE
        consts = np.concatenate(
            [
                drel_arr[:, wlo * T2 : whi * T2],
                vals_arr[:, wlo * T2 : whi * T2],
                iota_np,
                W1T,
                W2T,
                b1c,
                b2c,
            ],
            axis=1,
        )
        in_maps.append(
            {
                "x_full": x_tab,
                "ego_T": np.ascontiguousarray(ego_T_full[:, wlo * P : whi * P]),
                "idx16": np.ascontiguousarray(idx16[:, wlo * T2 * 8 : whi * T2 * 8]),
                "consts": np.ascontiguousarray(consts),
            }
        )
    return K_LO, K_HI, in_maps, perm, valid


def kernel(ego_embeddings, vals, W1, b1, W2, b2, rows, cols, _trace=False):
    K_LO, K_HI, in_maps, perm, valid = _prepare_inputs(
        ego_embeddings, vals, W1, b1, W2, b2, rows, cols
    )
    nc = _build_program(K_LO, K_HI)
    res = run_bass_kernel_spmd(nc, in_maps, list(range(N_CORES)), trace=_trace)
    out_T = np.concatenate([r["out_T"] for r in res.results], axis=1)
    out = np.empty((N_NODES, D), dtype=np.float32)
    out[perm[valid]] = out_T.T[valid]
    if _trace:
        kernel.last_exec_time_ns = res.exec_time_ns
        kernel.last_results = res
    return out
